# revision 16
# baseline (speedup 1.0000x reference)
"""FCOS post-processing kernel for Trainium2 (8 NeuronCores, data-parallel over batch).

Contract: kernel(**inputs) takes the full unsharded inputs from setup_inputs()
(logits{0..4}, reg{0..4}, ctr{0..4}, loc{0..4}) and returns the full outputs
(top_sc [16,256] f32, top_bx [16,256,4] f32, top_cl [16,256] i32, top_lv [16,256] i32),
matching reference.reference().

Algorithm (exact, per image):
  The reference's per-level top-1000 followed by global top-256 equals the global
  top-256 over ALL (level, location, class) candidates, since 256 <= 1000.
  Per image:
    A. location score M(hw) = sigmoid(max_c logit) * (max_c logit > tau) * sigmoid(ctr)
       (class-max commutes with sigmoid/threshold). Computed via PE transpose of
       [81,128] chunks (80 logit rows + ctr row) and DVE reduce-max.
    B. top-256 locations by M via gpsimd.topk (n=51200 zero-padded).
       Any candidate in the global top-256 lives in a top-256-by-M location
       (its own location max >= its score), so this pruning is exact.
    C. gather the 81-value columns (80 logits + ctr) of those 256 locations,
       recompute exact masked scores s = sig(l)*sig(ctr)*(l>tau) for 256x80.
    D. global top-256 = topk over the 20480 pruned candidate scores (sorted).
    E. decode: class = u%80, location via phase-B index table, level/stride by
       range compare, boxes from gathered reg/loc rows. All f32 ops IEEE-exact.
"""
import os
import uuid
import numpy as np

N_CORES = 8
N, C = 16, 80
SIZES = ((100, 152), (50, 76), (25, 38), (13, 19), (7, 10))
STRIDES = (8, 16, 32, 64, 128)
HWPAD = (15232, 3840, 1024, 256, 128)      # per-level padded to 128 multiple
HWV = sum(HWPAD)                            # 20480 virtual locations
LVL_STARTS = (0, 15232, 19072, 20096, 20352)
TAU = float(np.log(0.05 / 0.95))            # logit threshold for sigmoid > 0.05
NEG = -30.0                                 # pad logit: sigmoid ~ 0 < thresh
VOCAB = 51200                               # topk vocab (min allowed, 16*3200)
K = 256

_CACHE = {}


def _build_program():
    import concourse.bass as bass
    import concourse.bacc as bacc
    import concourse.tile as tile
    import concourse.mybir as mybir
    from concourse.tile_rust import add_dep_helper
    from contextlib import ExitStack

    AF = mybir.ActivationFunctionType
    ALU = mybir.AluOpType
    f32 = mybir.dt.float32
    u32 = mybir.dt.uint32
    i32 = mybir.dt.int32

    nc = bacc.Bacc()

    # ---- IO ----
    # build nonce: the XLA compile cache keys on the HLO (IO signature only),
    # not the embedded bass program — a unique input name forces a fresh NEFF.
    nonce_name = "nonce_" + uuid.uuid4().hex[:10]
    nonce_len = int.from_bytes(os.urandom(2), "little") % 4096 + 1
    _CACHE["nonce_name"] = nonce_name
    _CACHE["nonce_len"] = nonce_len
    nonce = nc.dram_tensor(nonce_name, [1, nonce_len], f32, kind="ExternalInput")
    lgc = nc.dram_tensor("lgc", [2, 81, HWV], f32, kind="ExternalInput")
    lgt = nc.dram_tensor("lgt", [2, HWV, 81], f32, kind="ExternalInput")   # transposed copy (phase C gather)
    rgl = nc.dram_tensor("rgl", [2, HWV, 6], f32, kind="ExternalInput")    # per-loc: l,t,r,b,locx,locy
    ident = nc.dram_tensor("ident", [128, 128], f32, kind="ExternalInput")
    DEBUG = bool(int(os.environ.get("BASS_FCOS_DEBUG", "0")))
    o_sc = nc.dram_tensor("o_sc", [2, 256], f32, kind="ExternalOutput")
    o_bx = nc.dram_tensor("o_bx", [2, 256, 4], f32, kind="ExternalOutput")
    o_cl = nc.dram_tensor("o_cl", [2, 256], i32, kind="ExternalOutput")
    o_lv = nc.dram_tensor("o_lv", [2, 256], i32, kind="ExternalOutput")

    NSTAGE = 20          # stages per image, 1024 locations each
    SPC = 8              # chunks (of 128 locs) per stage
    GRP = 4              # chunks per psum bank group

    with ExitStack() as ctx:
        tc = ctx.enter_context(tile.TileContext(nc))
        consts = ctx.enter_context(tc.tile_pool(name="consts", bufs=1))
        stage_p = ctx.enter_context(tc.tile_pool(name="stage", bufs=4))
        psum_p = ctx.enter_context(tc.tile_pool(name="psum", bufs=4, space="PSUM"))
        mtile_p = ctx.enter_context(tc.tile_pool(name="mtile", bufs=1))
        tk_p = ctx.enter_context(tc.tile_pool(name="tk", bufs=1))
        small_p = ctx.enter_context(tc.tile_pool(name="small", bufs=1))
        dram_p = ctx.enter_context(tc.tile_pool(name="dram", bufs=1, space="DRAM"))

        idn = consts.tile([128, 128], f32)
        nc.sync.dma_start(idn[:], ident[:])

        # per-image persistent tiles
        Mlg = [mtile_p.tile([128, 160], f32, tag=f"Mlg{i}", name=f"Mlg{i}") for i in range(2)]
        Sct = [mtile_p.tile([128, 160], f32, tag=f"Sct{i}", name=f"Sct{i}") for i in range(2)]
        tkB_in = [nc.alloc_sbuf_tensor(f"tkBi{i}", [16, 3200], f32) for i in range(2)]
        tkB_out = [nc.alloc_sbuf_tensor(f"tkBo{i}", [16, 32], u32) for i in range(2)]
        tkD_in = [nc.alloc_sbuf_tensor(f"tkDi{i}", [16, 3200], f32) for i in range(2)]
        tkD_out = [nc.alloc_sbuf_tensor(f"tkDo{i}", [16, 32], u32) for i in range(2)]
        scrM = [dram_p.tile([VOCAB], f32, tag=f"scrM{i}", name=f"scrM{i}") for i in range(2)]
        scrB = [dram_p.tile([256], u32, tag=f"scrB{i}", name=f"scrB{i}") for i in range(2)]
        scrS = [dram_p.tile([VOCAB], f32, tag=f"scrS{i}", name=f"scrS{i}") for i in range(2)]
        scrV = [dram_p.tile([256], f32, tag=f"scrV{i}", name=f"scrV{i}") for i in range(2)]
        scrU = [dram_p.tile([256], u32, tag=f"scrU{i}", name=f"scrU{i}") for i in range(2)]
        scrH = [dram_p.tile([256], u32, tag=f"scrH{i}", name=f"scrH{i}") for i in range(2)]

        # one-time pad zeroing of the DRAM scratch vocab tails
        zpad = small_p.tile([16, 1920], f32, tag="zpad", name="zpad")
        nc.vector.memset(zpad[:], 0.0)
        padw = []
        for i in range(2):
            for scr in (scrM[i], scrS[i]):
                padw.append(nc.sync.dma_start(
                    scr[HWV:VOCAB].rearrange("(a b) -> a b", a=16), zpad[:]))
        rbB, rbD = [None, None], [None, None]
        scatM, scatS = [None, None], [None, None]

        # ---------------- Phase A ----------------
        def phase_a(i):
            for st in range(NSTAGE):
                lgs = stage_p.tile([81, 1024], f32, tag="lgs")
                nc.sync.dma_start(lgs[:], lgc[i, :, st * 1024:(st + 1) * 1024])
                for g in range(SPC // GRP):
                    ps = psum_p.tile([128, GRP, 81], f32, tag="ps")
                    for jj in range(GRP):
                        col = (g * GRP + jj) * 128
                        nc.tensor.transpose(
                            ps[:, jj, :], lgs[:, col:col + 128], idn[0:81, 0:81])
                    j0 = st * 8 + g * GRP
                    # class-max over the 80 logit rows
                    nc.vector.tensor_reduce(
                        Mlg[i][:, j0:j0 + GRP],
                        ps[:, :, 0:80], axis=mybir.AxisListType.X, op=ALU.max)
                    # ctr column (transposed row 80)
                    nc.vector.tensor_copy(Sct[i][:, j0:j0 + GRP], ps[:, :, 80])
            # M = sig(maxlogit) * (maxlogit > tau) * sig(ctr)
            sm = small_p.tile([128, 160], f32, tag="sm")
            nc.scalar.activation(sm[:], Mlg[i][:], AF.Sigmoid)
            sc_ = small_p.tile([128, 160], f32, tag="sc_")
            nc.scalar.activation(sc_[:], Sct[i][:], AF.Sigmoid)
            msk = small_p.tile([128, 160], f32, tag="msk")
            nc.vector.scalar_tensor_tensor(
                msk[:], Mlg[i][:], TAU, sm[:], op0=ALU.is_gt, op1=ALU.mult)
            M = small_p.tile([128, 160], f32, tag="M")
            nc.vector.tensor_tensor(M[:], msk[:], sc_[:], op=ALU.mult)
            # bounce M -> DRAM (lambda order: addr = p*160 + j), reload [16, 0:1280]
            scatM[i] = nc.sync.dma_start(
                scrM[i][0:HWV].rearrange("(p j) -> p j", p=128), M[:])
            rbB[i] = nc.sync.dma_start(
                tkB_in[i][:], scrM[i][:].rearrange("(q x) -> q x", x=3200))
            add_dep_helper(rbB[i].ins, scatM[i].ins, reason="readback after scatter")
            for pw in padw:
                add_dep_helper(rbB[i].ins, pw.ins, reason="readback after pad zero")

        _dbg = {}

        # ---------------- Phase B ----------------
        def phase_b(i):
            if DEBUG and i == 0:
                nc.sync.dma_start(_dbg["tensors"]["d_M"][:], tkB_in[i][:])
            tkb = nc.gpsimd.topk(tkB_out[i][:], tkB_in[i][:], tokens=1, vocab_size=VOCAB, k=K)
            add_dep_helper(tkb.ins, rbB[i].ins, reason="topk B reads readback")
            if DEBUG and i == 0:
                dmp = nc.sync.dma_start(_dbg["tensors"]["d_tkB"][:], tkB_out[i][:])
                add_dep_helper(dmp.ins, tkb.ins, reason="dump after topk")
            # u_B [16,16] -> hw_v [16,16]; write scrB
            uf = small_p.tile([16, 16], f32, tag="uf")
            cp = nc.vector.tensor_copy(uf[:], tkB_out[i][:, 16:32])  # u32 -> f32
            add_dep_helper(cp.ins, tkb.ins, reason="read topk B out")
            # u = lambda directly (full-vocab layout); p = lam // 160
            lam = uf
            qi = small_p.tile([16, 16], i32, tag="qi")
            fx = small_p.tile([16, 16], f32, tag="fx")
            p = small_p.tile([16, 16], f32, tag="p")
            nc.vector.tensor_scalar_mul(p[:], lam[:], 1.0 / 160.0)
            nc.vector.tensor_copy(qi[:], p[:])
            nc.vector.tensor_copy(p[:], qi[:])
            nc.vector.scalar_tensor_tensor(
                fx[:], p[:], 160.0, lam[:], op0=ALU.mult, op1=ALU.is_gt)
            nc.vector.tensor_sub(p[:], p[:], fx[:])
            # hw = 128*(lam - 160*p) + p = 128*lam - 20480*p + p
            hw = small_p.tile([16, 16], f32, tag="hw")
            nc.vector.tensor_scalar_mul(hw[:], lam[:], 128.0)
            nc.vector.scalar_tensor_tensor(
                hw[:], p[:], -20479.0, hw[:], op0=ALU.mult, op1=ALU.add)
            hwu = small_p.tile([16, 16], u32, tag="hwu")
            nc.vector.tensor_copy(hwu[:], hw[:])
            nc.sync.dma_start(scrB[i][:].rearrange("(a b) -> a b", a=16), hwu[:])
            if DEBUG and i == 0:
                nc.sync.dma_start(_dbg["tensors"]["d_hwB"][:].rearrange("(a b) -> a b", a=16), hwu[:])

        # ---------------- Phase C ----------------
        def phase_c(i):
            iB = small_p.tile([128, 2], u32, tag="iB")
            nc.sync.dma_start(iB[:], scrB[i][:].rearrange("(t p) -> p t", p=128))
            G = small_p.tile([128, 2, 81], f32, tag="G")
            lgt_flat = lgt[:].rearrange("i hw c -> (i hw) c")
            for t in range(2):
                nc.gpsimd.indirect_dma_start(
                    out=G[:, t, :], out_offset=None, in_=lgt_flat,
                    in_offset=bass.IndirectOffsetOnAxis(ap=iB[:, t:t + 1], axis=0),
                    element_offset=i * HWV * 81)
            if DEBUG and i == 0:
                nc.sync.dma_start(_dbg["tensors"]["d_G"][:], G[:])
            sG = small_p.tile([128, 2, 81], f32, tag="sG")
            nc.scalar.activation(sG[:], G[:], AF.Sigmoid)
            mG = small_p.tile([128, 2, 80], f32, tag="mG")
            nc.vector.scalar_tensor_tensor(
                mG[:], G[:, :, 0:80], TAU, sG[:, :, 0:80], op0=ALU.is_gt, op1=ALU.mult)
            s = small_p.tile([128, 2, 80], f32, tag="s")
            nc.vector.tensor_tensor(
                s[:], mG[:], sG[:, :, 80:81].to_broadcast([128, 2, 80]), op=ALU.mult)
            # bounce: scrS[r, c] with r = t*128+p
            scatS[i] = nc.sync.dma_start(
                scrS[i][0:HWV].rearrange("(t p c) -> p t c", p=128, c=80), s[:])
            rbD[i] = nc.sync.dma_start(
                tkD_in[i][:], scrS[i][:].rearrange("(q x) -> q x", x=3200))
            add_dep_helper(rbD[i].ins, scatS[i].ins, reason="readback after scatter")
            for pw in padw:
                add_dep_helper(rbD[i].ins, pw.ins, reason="readback after pad zero")

        # ---------------- Phase D + E ----------------
        def phase_de(i):
            if DEBUG and i == 0:
                nc.sync.dma_start(_dbg["tensors"]["d_s"][:], tkD_in[i][:])
            tkd = nc.gpsimd.topk(tkD_out[i][:], tkD_in[i][:], tokens=1, vocab_size=VOCAB, k=K)
            add_dep_helper(tkd.ins, rbD[i].ins, reason="topk D reads readback")
            if DEBUG and i == 0:
                dmp = nc.sync.dma_start(_dbg["tensors"]["d_tkD"][:], tkD_out[i][:])
                add_dep_helper(dmp.ins, tkd.ins, reason="dump after topk")
            # bounce vals and u to [128, 2] layout (r = t*128 + p)
            vw = small_p.tile([16, 16], f32, tag="vw")
            cpv = nc.vector.tensor_copy(vw[:], tkD_out[i][:, 0:16].bitcast(f32))
            add_dep_helper(cpv.ins, tkd.ins, reason="read topk D vals")
            nc.sync.dma_start(scrV[i][:].rearrange("(a b) -> a b", a=16), vw[:])
            dmu = nc.sync.dma_start(scrU[i][:].rearrange("(a b) -> a b", a=16), tkD_out[i][:, 16:32])
            add_dep_helper(dmu.ins, tkd.ins, reason="read topk D idx")
            V = small_p.tile([128, 2], f32, tag="V")
            nc.sync.dma_start(V[:], scrV[i][:].rearrange("(t p) -> p t", p=128))
            U = small_p.tile([128, 2], u32, tag="U")
            nc.sync.dma_start(U[:], scrU[i][:].rearrange("(t p) -> p t", p=128))

            uf = small_p.tile([128, 2], f32, tag="euf")
            nc.vector.tensor_copy(uf[:], U[:])
            # rB = u // 80 (fixup) ; c = u - 80*rB
            rB = small_p.tile([128, 2], f32, tag="erB")
            nc.vector.tensor_scalar_mul(rB[:], uf[:], 1.0 / 80.0)
            qi = small_p.tile([128, 2], i32, tag="eqi")
            nc.vector.tensor_copy(qi[:], rB[:])
            nc.vector.tensor_copy(rB[:], qi[:])
            fx = small_p.tile([128, 2], f32, tag="efx")
            nc.vector.scalar_tensor_tensor(
                fx[:], rB[:], 80.0, uf[:], op0=ALU.mult, op1=ALU.is_gt)
            nc.vector.tensor_sub(rB[:], rB[:], fx[:])
            cc = small_p.tile([128, 2], f32, tag="ecc")
            nc.vector.scalar_tensor_tensor(
                cc[:], rB[:], -80.0, uf[:], op0=ALU.mult, op1=ALU.add)
            nc.vector.tensor_scalar_min(rB[:], rB[:], 255.0)
            rBu = small_p.tile([128, 2], u32, tag="erBu")
            nc.vector.tensor_copy(rBu[:], rB[:])
            # hw_v = scrB[rB]
            hw = small_p.tile([128, 2], u32, tag="ehw")
            for t in range(2):
                nc.gpsimd.indirect_dma_start(
                    out=hw[:, t:t + 1], out_offset=None,
                    in_=scrB[i][:].rearrange("(a b) -> a b", b=1),
                    in_offset=bass.IndirectOffsetOnAxis(ap=rBu[:, t:t + 1], axis=0))
            nc.vector.tensor_scalar_min(hw[:], hw[:], HWV - 1)
            if DEBUG and i == 0:
                nc.sync.dma_start(_dbg["tensors"]["d_hwE"][:], hw[:])
            # gather reg/loc rows
            RG = small_p.tile([128, 2, 6], f32, tag="RG")
            rgl_flat = rgl[:].rearrange("i hw c -> (i hw) c")
            for t in range(2):
                nc.gpsimd.indirect_dma_start(
                    out=RG[:, t, :], out_offset=None, in_=rgl_flat,
                    in_offset=bass.IndirectOffsetOnAxis(ap=hw[:, t:t + 1], axis=0),
                    element_offset=i * HWV * 6)
            if DEBUG and i == 0:
                nc.sync.dma_start(_dbg["tensors"]["d_RG"][:], RG[:])
            # level + stride from hw
            hwf = small_p.tile([128, 2], f32, tag="ehwf")
            nc.vector.tensor_copy(hwf[:], hw[:])
            lvl = small_p.tile([128, 2], f32, tag="elvl")
            str_ = small_p.tile([128, 2], f32, tag="estr")
            m = small_p.tile([128, 2], f32, tag="em")
            nc.vector.memset(lvl[:], 0.0)
            nc.vector.memset(str_[:], 8.0)
            for k, (b, add) in enumerate(zip(LVL_STARTS[1:], (8.0, 16.0, 32.0, 64.0))):
                nc.vector.tensor_scalar(
                    m[:], hwf[:], float(b), scalar2=None, op0=ALU.is_ge)
                nc.vector.tensor_add(lvl[:], lvl[:], m[:])
                nc.vector.scalar_tensor_tensor(
                    str_[:], m[:], add, str_[:], op0=ALU.mult, op1=ALU.add)
            # boxes: x1 = locx - l*s ; y1 = locy - t*s ; x2 = locx + r*s ; y2 = locy + b*s
            bx = small_p.tile([128, 2, 4], f32, tag="ebx")
            rs = small_p.tile([128, 2], f32, tag="ers")
            for k, (ch, lc, sgn) in enumerate(((0, 4, -1.0), (1, 5, -1.0), (2, 4, 1.0), (3, 5, 1.0))):
                nc.vector.tensor_tensor(rs[:], RG[:, :, ch], str_[:], op=ALU.mult)
                if sgn < 0:
                    nc.vector.tensor_sub(bx[:, :, k], RG[:, :, lc], rs[:])
                else:
                    nc.vector.tensor_add(bx[:, :, k], RG[:, :, lc], rs[:])
            # mask for empty slots (vals <= 0): zero boxes/scores
            posm = small_p.tile([128, 2], f32, tag="eposm")
            nc.vector.tensor_scalar(
                posm[:], V[:], 0.0, scalar2=None, op0=ALU.is_gt)
            sq = small_p.tile([128, 2], f32, tag="esq")
            nc.scalar.activation(sq[:], V[:], AF.Sqrt)
            nc.vector.tensor_tensor(sq[:], sq[:], posm[:], op=ALU.mult)
            for k in range(4):
                nc.vector.tensor_tensor(bx[:, :, k], bx[:, :, k], posm[:], op=ALU.mult)
            cli = small_p.tile([128, 2], i32, tag="ecli")
            nc.vector.tensor_copy(cli[:], cc[:])
            lvi = small_p.tile([128, 2], i32, tag="elvi")
            nc.vector.tensor_copy(lvi[:], lvl[:])
            # outputs (ascending rank; host reverses)
            nc.sync.dma_start(
                bass.AP(o_sc, i * 256, [[1, 128], [128, 2]]), sq[:])
            nc.sync.dma_start(
                bass.AP(o_cl, i * 256, [[1, 128], [128, 2]]), cli[:])
            nc.sync.dma_start(
                bass.AP(o_lv, i * 256, [[1, 128], [128, 2]]), lvi[:])
            nc.sync.dma_start(
                bass.AP(o_bx, i * 1024, [[4, 128], [512, 2], [1, 4]]), bx[:])

        REPEAT = int(os.environ.get("BASS_FCOS_REPEAT", "1"))
        if DEBUG:
            d_M = nc.dram_tensor("d_M", [16, 3200], f32, kind="ExternalOutput")
            d_tkB = nc.dram_tensor("d_tkB", [16, 32], u32, kind="ExternalOutput")
            d_hwB = nc.dram_tensor("d_hwB", [256], u32, kind="ExternalOutput")
            d_G = nc.dram_tensor("d_G", [128, 2, 81], f32, kind="ExternalOutput")
            d_s = nc.dram_tensor("d_s", [16, 3200], f32, kind="ExternalOutput")
            d_tkD = nc.dram_tensor("d_tkD", [16, 32], u32, kind="ExternalOutput")
            d_hwE = nc.dram_tensor("d_hwE", [128, 2], u32, kind="ExternalOutput")
            d_RG = nc.dram_tensor("d_RG", [128, 2, 6], f32, kind="ExternalOutput")
            _dbg["tensors"] = dict(d_M=d_M, d_tkB=d_tkB, d_hwB=d_hwB, d_G=d_G,
                                   d_s=d_s, d_tkD=d_tkD, d_hwE=d_hwE, d_RG=d_RG)

        def whole():
            for i in range(2):
                phase_a(i)
                phase_b(i)
                phase_c(i)
                phase_de(i)

        if REPEAT > 1:
            with tc.For_i(0, REPEAT, 1):
                whole()
        else:
            whole()

    nc.finalize()
    return nc


def _prep_core_inputs(inputs, core):
    """Build per-core host-side input map (layout transforms only)."""
    i0 = core * 2
    lgc = np.full((2, 81, HWV), NEG, dtype=np.float32)
    rglt = np.zeros((2, HWV, 6), dtype=np.float32)
    for l in range(5):
        h, w = SIZES[l]
        hw = h * w
        s0 = LVL_STARTS[l]
        lg = np.asarray(inputs[f'logits{l}'][i0:i0 + 2], dtype=np.float32)
        ct = np.asarray(inputs[f'ctr{l}'][i0:i0 + 2], dtype=np.float32)
        rg = np.asarray(inputs[f'reg{l}'][i0:i0 + 2], dtype=np.float32)
        lc = np.asarray(inputs[f'loc{l}'], dtype=np.float32)
        lgc[:, 0:80, s0:s0 + hw] = lg.reshape(2, 80, hw)
        lgc[:, 80, s0:s0 + hw] = ct.reshape(2, hw)
        rglt[:, s0:s0 + hw, 0:4] = rg.reshape(2, 4, hw).transpose(0, 2, 1)
        rglt[:, s0:s0 + hw, 4:6] = lc[None, :, :]
    lgt = np.ascontiguousarray(lgc.transpose(0, 2, 1))
    return {
        _CACHE.get("nonce_name", "nonce"): np.zeros((1, _CACHE.get("nonce_len", 1)), np.float32),
        "lgc": lgc,
        "lgt": lgt,
        "rgl": rglt,
        "ident": np.eye(128, dtype=np.float32),
    }


def kernel(**inputs):
    from concourse.bass_utils import run_bass_kernel_spmd

    if "nc" not in _CACHE:
        _CACHE["nc"] = _build_program()
    nc = _CACHE["nc"]

    in_maps = [_prep_core_inputs(inputs, c) for c in range(N_CORES)]
    res = run_bass_kernel_spmd(nc, in_maps, list(range(N_CORES)))

    sc = np.concatenate([r["o_sc"] for r in res.results], axis=0)   # [16, 256] asc
    bx = np.concatenate([r["o_bx"] for r in res.results], axis=0)
    cl = np.concatenate([r["o_cl"] for r in res.results], axis=0)
    lv = np.concatenate([r["o_lv"] for r in res.results], axis=0)
    # device produced ascending rank; reference wants descending
    sc = np.ascontiguousarray(sc[:, ::-1])
    bx = np.ascontiguousarray(bx[:, ::-1])
    cl = np.ascontiguousarray(cl[:, ::-1]).astype(np.int32)
    lv = np.ascontiguousarray(lv[:, ::-1]).astype(np.int32)
    return sc, bx, cl, lv
